# revision 11
# baseline (speedup 1.0000x reference)
"""Trainium2 Bass kernel for a small MoE layer (4 routed top-2 + 2 shared experts).

Strategy: UNIFIED EXPERT-PARALLEL over 8 NeuronCores, all matmuls in fp8-e4m3
DoubleRow perf mode (256-deep contraction, 0.5 cycles/row -> 4x bf16 matmul
throughput in the TRN2 cost model).

Every (token, expert) pair becomes a unit of work: shared experts are "routed
with scale 0.5" for every token; routed experts use the host-computed top-2
selection (same fp32 math as the reference).  Totals: 2*8192 shared pairs +
16384 routed pairs = 32768.  With this seed the per-expert counts are
[4157, 4137, 4013, 4077] routed + 4096-token halves of the shared experts, so
EVERY CORE SERVES EXACTLY ONE EXPERT with capacity 4224 tokens (16 blocks of
256 + 1 of 128):
  core 0/1: shared expert 0 (tokens 0..4095 / 4096..8191)
  core 2/3: shared expert 1
  core 4+r: routed expert r (its selected tokens, host-gathered)

Numerics (rel err ~2e-3 vs fp32 reference, gate is 2e-2): e4m3 alone is too
coarse (each of x/w1/h/w2 alone gives ~2.4e-2), so every operand is split
base+residual at the SAME power-of-2 scale (residual relies on e4m3
subnormals) and the kernel runs 6 fp8 passes:
  L1: xq@w1q + xlo@w1q + xq@w1l          (pairs of 128-deep k-tiles)
  L2: hq@w2q + hlo@w2q + hq@w2l
  h  = relu(psum)*2^-6 in bf16 (ACT), hq = e4m3(h) (ACT), hlo = h - hq (DVE)
Gating runs on-device in transposed [E, tok] layout from the same fp8 x
(+ residual passes), exp on ACT, then a tiny f32 matmul against per-core
selector columns [sel*2^-10, ones] gives num/den per token; DVE reciprocal +
multiply yields the per-token drain scale (0.5*2^-10 for shared cores, softmax
prob * 2^-10 for routed).  The L2 drain is a DVE tensor_scalar multiply by
that [128,1] scale (psum carries 2^10 scaling), output bf16, host scatter-adds.

HW constraint (walrus 's3_lw_dual_fp8_restrictions'): a DoubleRow Ldweights
needs its two fp8 k-tiles CONTIGUOUS per partition with 128 active columns, so
every stationary operand is laid out pair-packed:
  w1 [128, 8(dpair: 4 base + 4 resid), 64(fchunk x2 ktile), 128]
  hq/hlo [128, 32(fpair x2 tokchunk), 2(ktile), 128]
  gw padded to 128 columns (experts in cols 0..3)
The moving operand (x, w2) has no such restriction.

Scales: x*16, w*64 -> L1 psum = 1024*x@w1; h stored as 16*relu(.) so L2 psum =
1024*h@w2; all residuals quantized at the base tensor's scale so every pass
accumulates into the same PSUM scale.
"""

import sys

sys.path.insert(0, '/opt/trn_rl_repo')

import numpy as np
import ml_dtypes

import concourse.bass as bass
import concourse.mybir as mybir
import concourse.tile as tile
from concourse import bacc
from concourse.bass_utils import run_bass_kernel_spmd

E4 = ml_dtypes.float8_e4m3
BF16 = ml_dtypes.bfloat16

NCORES = 8
B, S, D, F, O = 4, 2048, 1024, 4096, 1024
E, NS = 4, 2
T = B * S                 # 8192 tokens
P = 128
DK = D // P               # 8 k-tiles over D
DK2 = DK // 2             # 4 d k-tile pairs
FCH = F // P              # 32 f-chunks
FK2 = FCH // 2            # 16 f k-tile pairs

_CACHED = {}


def _build(blks, b1nz=False):
    """blks: tuple of per-block token counts, e.g. (256,)*16 + (128,)."""
    f32 = mybir.dt.float32
    bf = mybir.dt.bfloat16
    f8 = mybir.dt.float8e4
    AF = mybir.ActivationFunctionType
    ALU = mybir.AluOpType
    DR = mybir.MatmulPerfMode.DoubleRow
    NBLK = len(blks)
    BMAX = max(blks)

    nc = bacc.Bacc("TRN2", target_bir_lowering=False, debug=False)

    xq_d = nc.dram_tensor("xq", [NBLK, P, DK, BMAX], f8, kind="ExternalInput")
    xlo_d = nc.dram_tensor("xlo", [NBLK, P, DK, BMAX], f8, kind="ExternalInput")
    w1_d = nc.dram_tensor("w1", [P, 2 * DK2, 2 * FCH, P], f8,
                          kind="ExternalInput")
    w2_d = nc.dram_tensor("w2", [P, 2 * FCH, O], f8, kind="ExternalInput")
    b1_d = nc.dram_tensor("b1", [P, FCH], f32, kind="ExternalInput")  # 16*b1
    gw_d = nc.dram_tensor("gw", [P, 2 * DK, P], f8, kind="ExternalInput")
    gb_d = nc.dram_tensor("gb", [E, 1], f32, kind="ExternalInput")
    sel_d = nc.dram_tensor("sel", [E, 2], f32, kind="ExternalInput")
    yg_d = nc.dram_tensor("yg", [NBLK, BMAX, O], bf, kind="ExternalOutput")

    with tile.TileContext(nc) as tc:
        with (
            tc.tile_pool(name="wres", bufs=1) as wres,
            tc.tile_pool(name="consts", bufs=1) as consts,
            tc.tile_pool(name="xp", bufs=3) as xp,
            tc.tile_pool(name="xlp", bufs=3) as xlp,
            tc.tile_pool(name="hqp", bufs=2) as hqp,
            tc.tile_pool(name="hlp", bufs=2) as hlp,
            tc.tile_pool(name="hfp", bufs=3) as hfp,
            tc.tile_pool(name="gep", bufs=2) as gep,
            tc.tile_pool(name="sp", bufs=8) as sp,
            tc.tile_pool(name="outp", bufs=3) as outp,
            tc.tile_pool(name="l1ps", bufs=2, space="PSUM") as l1ps,
            tc.tile_pool(name="l2ps", bufs=4, space="PSUM") as l2ps,
            tc.tile_pool(name="gps", bufs=2, space="PSUM") as gps,
        ):
            # ---- consts first (tiny), then x(0), w1, x(1), w2: this DMA
            # issue order keeps the PE fed from ~2us.
            b1t = consts.tile([P, FCH], f32, tag="b1t", name="b1t")
            nc.sync.dma_start(b1t[:], b1_d[:, :])
            gwt = consts.tile([P, 2 * DK, P], f8, tag="gwt", name="gwt")
            nc.sync.dma_start(gwt[:], gw_d[:, :, :])
            gbt = consts.tile([E, 1], f32, tag="gbt", name="gbt")
            nc.sync.dma_start(gbt[:], gb_d[:, :])
            selt = consts.tile([E, 2], f32, tag="selt", name="selt")
            nc.sync.dma_start(selt[:], sel_d[:, :])

            hq_t = {}
            hlo_t = {}
            s_t = {}
            blk_x = {}

            def fetch_x(blk):
                Tb = blks[blk]
                xqt = xp.tile([P, DK, BMAX], f8, tag="xqt", name=f"xqt_{blk}")
                nc.sync.dma_start(xqt[:, :, :Tb], xq_d[blk, :, :, :Tb])
                xlt = xlp.tile([P, DK, BMAX], f8, tag="xlt", name=f"xlt_{blk}")
                nc.sync.dma_start(xlt[:, :, :Tb], xlo_d[blk, :, :, :Tb])
                blk_x[blk] = (xqt, xlt)

            fetch_x(0)
            w1t = wres.tile([P, 2 * DK2, 2 * FCH, P], f8, tag="w1t",
                            name="w1t")
            for c in range(8):
                csl = slice(c * 8, (c + 1) * 8)
                nc.sync.dma_start(w1t[:, :, csl, :], w1_d[:, :, csl, :])
            if NBLK > 1:
                fetch_x(1)
            w2t = wres.tile([P, 2 * FCH, O], f8, tag="w2t", name="w2t")
            for c in range(8):
                nc.sync.dma_start(w2t[:, c * 8:(c + 1) * 8, :],
                                  w2_d[:, c * 8:(c + 1) * 8, :])

            def emit_l2(blk):
                Tb = blks[blk]
                hq, hlo = hq_t[blk], hlo_t[blk]
                for tcg in range(Tb // P):
                    outt = outp.tile([P, O], bf, tag="outt",
                                     name=f"outt_{blk}_{tcg}")
                    for oh in range(2):
                        yp = l2ps.tile([P, 512], f32, tag="yp",
                                       name=f"yp_{blk}_{tcg}_{oh}")
                        osl = slice(oh * 512, (oh + 1) * 512)
                        for jp in range(FK2):
                            nc.tensor.matmul(
                                yp[:], lhsT=hq[:, 2 * jp + tcg, :, :],
                                rhs=w2t[:, 2 * jp:2 * jp + 2, osl],
                                perf_mode=DR, start=(jp == 0), stop=False)
                        for jp in range(FK2):
                            nc.tensor.matmul(
                                yp[:], lhsT=hlo[:, 2 * jp + tcg, :, :],
                                rhs=w2t[:, 2 * jp:2 * jp + 2, osl],
                                perf_mode=DR, start=False, stop=False)
                        for jp in range(FK2):
                            nc.tensor.matmul(
                                yp[:], lhsT=hq[:, 2 * jp + tcg, :, :],
                                rhs=w2t[:, 2 * FCH - 2 * FK2 + 2 * jp:
                                        2 * FCH - 2 * FK2 + 2 * jp + 2, osl],
                                perf_mode=DR, start=False, stop=(jp == FK2 - 1))
                        nc.vector.tensor_scalar(
                            outt[:, osl], yp[:], s_t[(blk, tcg)][:, 0:1], None,
                            ALU.mult)
                    nc.sync.dma_start(yg_d[blk, tcg * P:(tcg + 1) * P, :],
                                      outt[:])

            for blk in range(NBLK):
                Tb = blks[blk]
                ntc = Tb // P
                if blk not in blk_x:
                    fetch_x(blk)
                xqt, xlt = blk_x[blk]

                # ---- gating: logits^T [E(+pad), Tb] at scale 256 ----
                gpsu = gps.tile([P, BMAX], f32, tag="gpsu", name=f"gpsu_{blk}",
                                bufs=1)
                for j in range(DK2):
                    nc.tensor.matmul(
                        gpsu[:, :Tb], lhsT=gwt[:, 2 * j:2 * j + 2, :],
                        rhs=xqt[:, 2 * j:2 * j + 2, :Tb],
                        perf_mode=DR, start=(j == 0), stop=False)
                for j in range(DK2):
                    nc.tensor.matmul(
                        gpsu[:, :Tb], lhsT=gwt[:, 2 * j:2 * j + 2, :],
                        rhs=xlt[:, 2 * j:2 * j + 2, :Tb],
                        perf_mode=DR, start=False, stop=False)
                for j in range(DK2):
                    nc.tensor.matmul(
                        gpsu[:, :Tb], lhsT=gwt[:, DK + 2 * j:DK + 2 * j + 2, :],
                        rhs=xqt[:, 2 * j:2 * j + 2, :Tb],
                        perf_mode=DR, start=False, stop=(j == DK2 - 1))
                gext = gep.tile([E, BMAX], f32, tag="gext", name=f"gext_{blk}")
                nc.scalar.activation(gext[:, :Tb], gpsu[:E, :Tb], AF.Exp,
                                     bias=gbt[:, 0:1], scale=1.0 / 256.0)
                ndt = gps.tile([P, 2, 2], f32, tag="ndt", name=f"ndt_{blk}",
                               bufs=1)
                for tcg in range(ntc):
                    nc.tensor.matmul(
                        ndt[:, tcg, :], lhsT=gext[:, tcg * P:(tcg + 1) * P],
                        rhs=selt[:], start=True, stop=True)
                    rt = sp.tile([P, 1], f32, tag="rt", name=f"rt_{blk}_{tcg}")
                    nc.vector.reciprocal(rt[:], ndt[:, tcg, 1:2])
                    st = sp.tile([P, 1], f32, tag="st", name=f"st_{blk}_{tcg}")
                    nc.vector.tensor_tensor(st[:], ndt[:, tcg, 0:1], rt[:],
                                            ALU.mult)
                    s_t[(blk, tcg)] = st

                # ---- L1: h for all of F, pair-packed for L2 ldweights ----
                # hq/hlo [128, 32(=fpair*2+tokchunk), 2(f ktile), 128(tok)]
                hq = hqp.tile([P, FCH, 2, P], f8, tag="hq", name=f"hq_{blk}")
                hlo = hlp.tile([P, FCH, 2, P], f8, tag="hlo", name=f"hlo_{blk}")
                hq_t[blk], hlo_t[blk] = hq, hlo
                for fcp in range(FCH // 2):
                    ps = l1ps.tile([P, 2, 2, P], f32, tag="l1t",
                                   name=f"l1t_{blk}_{fcp}")
                    for half in range(2):
                        fc = 2 * fcp + half
                        pso = ps[:, half, :, :] if ntc == 2 else ps[:, half, 0, :]
                        for j in range(DK2):
                            nc.tensor.matmul(
                                pso, lhsT=w1t[:, j, 2 * fc:2 * fc + 2, :],
                                rhs=xqt[:, 2 * j:2 * j + 2, :Tb],
                                perf_mode=DR, start=(j == 0), stop=False)
                        for j in range(DK2):
                            nc.tensor.matmul(
                                pso, lhsT=w1t[:, j, 2 * fc:2 * fc + 2, :],
                                rhs=xlt[:, 2 * j:2 * j + 2, :Tb],
                                perf_mode=DR, start=False, stop=False)
                        for j in range(DK2):
                            nc.tensor.matmul(
                                pso,
                                lhsT=w1t[:, DK2 + j, 2 * fc:2 * fc + 2, :],
                                rhs=xqt[:, 2 * j:2 * j + 2, :Tb],
                                perf_mode=DR, start=False,
                                stop=(j == DK2 - 1))
                    hf = hfp.tile([P, 2, 2, P], bf, tag="hf",
                                  name=f"hf_{blk}_{fcp}")
                    # hf = relu(psum*2^-10 + b1)*16 == relu(psum*2^-6 + 16*b1)
                    if b1nz:
                        for half in range(2):
                            fc = 2 * fcp + half
                            nc.scalar.activation(
                                hf[:, half, :ntc, :], ps[:, half, :ntc, :],
                                AF.Relu, bias=b1t[:, fc:fc + 1],
                                scale=1.0 / 64.0)
                    else:
                        nc.scalar.activation(hf[:, :, :ntc, :],
                                             ps[:, :, :ntc, :], AF.Relu,
                                             bias=0.0, scale=1.0 / 64.0)
                    for half in range(2):
                        dsl = (slice(None), slice(2 * fcp, 2 * fcp + ntc),
                               half, slice(None))
                        nc.scalar.activation(hq[dsl], hf[:, half, :ntc, :],
                                             AF.Copy, bias=0.0, scale=1.0)
                        nc.vector.tensor_tensor(hlo[dsl], hf[:, half, :ntc, :],
                                                hq[dsl], ALU.subtract)

                if blk > 0:
                    emit_l2(blk - 1)
                    del hq_t[blk - 1], hlo_t[blk - 1], blk_x[blk - 1]
            emit_l2(NBLK - 1)

    nc.finalize()
    return nc


def _get_nc(blks, b1nz=False):
    key = (tuple(blks), b1nz)
    if key not in _CACHED:
        _CACHED[key] = _build(tuple(blks), b1nz)
    return _CACHED[key]


def _quant_pair(a, scale):
    """(base, residual) e4m3 value arrays for a*scale, residual at same scale."""
    hi = (a * scale).astype(E4)
    lo = (a * scale - hi.astype(np.float32)).astype(E4)
    return hi, lo


def kernel(**inputs) -> np.ndarray:
    x = np.asarray(inputs['x'], np.float32).reshape(T, D)
    gw = np.asarray(inputs['gate_w'], np.float32)
    gb = np.asarray(inputs['gate_b'], np.float32)
    sw1 = np.asarray(inputs['sw1'], np.float32)
    sb1 = np.asarray(inputs['sb1'], np.float32)
    sw2 = np.asarray(inputs['sw2'], np.float32)
    sb2 = np.asarray(inputs['sb2'], np.float32)
    rw1 = np.asarray(inputs['rw1'], np.float32)
    rb1 = np.asarray(inputs['rb1'], np.float32)
    rw2 = np.asarray(inputs['rw2'], np.float32)
    rb2 = np.asarray(inputs['rb2'], np.float32)
    assert not (np.any(sb2) or np.any(rb2)), "nonzero second-layer bias unsupported"

    # host gating (same fp32 math as reference) -> top-2 masks
    logits = x @ gw + gb
    m1 = logits.max(1, keepdims=True)
    pm = logits + (logits >= m1) * np.float32(-1e30)
    keep = logits >= pm.max(1, keepdims=True)
    counts = keep.sum(0)

    half = T // 2
    cap_needed = max(half, int(counts.max()))
    nfull, rem = divmod(cap_needed, 256)
    if rem == 0:
        blks = (256,) * nfull
    elif rem <= 128:
        blks = (256,) * nfull + (128,)
    else:
        blks = (256,) * (nfull + 1)
    cap = sum(blks)
    NBLK, BMAX = len(blks), max(blks)

    xq8, xlo8 = _quant_pair(x, np.float32(16.0))

    w1_all = np.concatenate([sw1, rw1], axis=0)
    w2_all = np.concatenate([sw2, rw2], axis=0)
    b1_all = np.concatenate([sb1, rb1], axis=0)
    w1p, w2p, b1p = [], [], []
    for e in range(NS + E):
        h1, l1 = _quant_pair(w1_all[e], np.float32(64.0))
        # [128, dpair(4 base + 4 resid), fchunk*2(ktile-in-pair), 128]
        packs = []
        for arr in (h1, l1):
            a5 = arr.reshape(DK2, 2, P, FCH, P)        # [jp, i, p, fc, c]
            packs.append(a5.transpose(2, 0, 3, 1, 4).reshape(P, DK2, 2 * FCH, P))
        w1p.append(np.ascontiguousarray(np.concatenate(packs, axis=1)))
        h2, l2 = _quant_pair(w2_all[e], np.float32(64.0))
        w2cat = np.concatenate([
            h2.reshape(FCH, P, O), l2.reshape(FCH, P, O)], axis=0)
        w2p.append(np.ascontiguousarray(w2cat.transpose(1, 0, 2)))
        b1p.append(np.ascontiguousarray(
            (16.0 * b1_all[e]).reshape(FCH, P).T.astype(np.float32)))
    gwh, gwl = _quant_pair(gw, np.float32(16.0))
    gwpad = np.zeros((2 * DK, P, P), E4)               # [ktile, p, col(pad 128)]
    gwpad[:DK, :, :E] = gwh.reshape(DK, P, E)
    gwpad[DK:, :, :E] = gwl.reshape(DK, P, E)
    gwp = np.ascontiguousarray(gwpad.transpose(1, 0, 2))
    gbp = gb.reshape(E, 1).astype(np.float32)

    core_expert = [0, 0, 1, 1, 2, 3, 4, 5]
    core_tokens = [
        np.arange(0, half), np.arange(half, T),
        np.arange(0, half), np.arange(half, T),
    ] + [np.nonzero(keep[:, r])[0] for r in range(E)]

    def xpack(tl, src):
        arr = np.zeros((cap, D), E4)
        arr[:len(tl)] = src[tl]
        out = np.zeros((NBLK, P, DK, BMAX), E4)
        t0 = 0
        for i, tb in enumerate(blks):
            blkarr = arr[t0:t0 + tb].astype(np.float32)  # [tb, D]
            out[i, :, :, :tb] = blkarr.reshape(tb, DK, P).transpose(2, 1, 0)
            t0 += tb
        return out

    in_maps = []
    for c in range(NCORES):
        e = core_expert[c]
        tl = core_tokens[c]
        assert len(tl) <= cap
        sel = np.zeros((E, 2), np.float32)
        sel[:, 1] = 1.0
        if c < 4:
            sel[:, 0] = 0.5 * (2.0 ** -10)
        else:
            sel[c - 4, 0] = 2.0 ** -10
        in_maps.append({
            "xq": xpack(tl, xq8), "xlo": xpack(tl, xlo8),
            "w1": w1p[e], "w2": w2p[e], "b1": b1p[e],
            "gw": gwp, "gb": gbp, "sel": sel,
        })

    b1nz = bool(np.any(sb1) or np.any(rb1))
    nc = _get_nc(blks, b1nz)
    res = run_bass_kernel_spmd(nc, in_maps, list(range(NCORES)))

    out = np.zeros((T, O), np.float32)
    for c in range(NCORES):
        tl = core_tokens[c]
        yg = np.asarray(res.results[c]["yg"]).reshape(NBLK, BMAX, O)
        yg = np.concatenate([yg[i, :tb] for i, tb in enumerate(blks)], axis=0)
        out[tl] += yg[:len(tl)].astype(np.float32)
        # general-case second-layer bias would be added here (zero for this net)
    return out.reshape(B, S, O)


# revision 12
# speedup vs baseline: 1.0250x; 1.0250x over previous
"""Trainium2 Bass kernel for a small MoE layer (4 routed top-2 + 2 shared experts).

Strategy: UNIFIED EXPERT-PARALLEL over 8 NeuronCores, all matmuls in fp8-e4m3
DoubleRow perf mode (256-deep contraction, 0.5 cycles/row -> 4x bf16 matmul
throughput in the TRN2 cost model).

Every (token, expert) pair becomes a unit of work: shared experts are "routed
with scale 0.5" for every token; routed experts use the host-computed top-2
selection (same fp32 math as the reference).  Totals: 2*8192 shared pairs +
16384 routed pairs = 32768.  With this seed the per-expert counts are
[4157, 4137, 4013, 4077] routed + 4096-token halves of the shared experts, so
EVERY CORE SERVES EXACTLY ONE EXPERT with capacity 4224 tokens (16 blocks of
256 + 1 of 128):
  core 0/1: shared expert 0 (tokens 0..4095 / 4096..8191)
  core 2/3: shared expert 1
  core 4+r: routed expert r (its selected tokens, host-gathered)

Numerics (rel err ~2e-3 vs fp32 reference, gate is 2e-2): e4m3 alone is too
coarse (each of x/w1/h/w2 alone gives ~2.4e-2), so every operand is split
base+residual at the SAME power-of-2 scale (residual relies on e4m3
subnormals) and the kernel runs 6 fp8 passes:
  L1: xq@w1q + xlo@w1q + xq@w1l          (pairs of 128-deep k-tiles)
  L2: hq@w2q + hlo@w2q + hq@w2l
  h  = relu(psum)*2^-6 in bf16 (ACT), hq = e4m3(h) (ACT), hlo = h - hq (DVE)
Gating runs on-device in transposed [E, tok] layout from the same fp8 x
(+ residual passes), exp on ACT, then a tiny f32 matmul against per-core
selector columns [sel*2^-10, ones] gives num/den per token; DVE reciprocal +
multiply yields the per-token drain scale (0.5*2^-10 for shared cores, softmax
prob * 2^-10 for routed).  The L2 drain is a DVE tensor_scalar multiply by
that [128,1] scale (psum carries 2^10 scaling), output bf16, host scatter-adds.

HW constraint (walrus 's3_lw_dual_fp8_restrictions'): a DoubleRow Ldweights
needs its two fp8 k-tiles CONTIGUOUS per partition with 128 active columns, so
every stationary operand is laid out pair-packed:
  w1 [128, 8(dpair: 4 base + 4 resid), 64(fchunk x2 ktile), 128]
  hq/hlo [128, 32(fpair x2 tokchunk), 2(ktile), 128]
  gw padded to 128 columns (experts in cols 0..3)
The moving operand (x, w2) has no such restriction.

Scales: x*16, w*64 -> L1 psum = 1024*x@w1; h stored as 16*relu(.) so L2 psum =
1024*h@w2; all residuals quantized at the base tensor's scale so every pass
accumulates into the same PSUM scale.
"""

import sys

sys.path.insert(0, '/opt/trn_rl_repo')

import numpy as np
import ml_dtypes

import concourse.bass as bass
import concourse.mybir as mybir
import concourse.tile as tile
from concourse import bacc
from concourse.bass_utils import run_bass_kernel_spmd

E4 = ml_dtypes.float8_e4m3
BF16 = ml_dtypes.bfloat16

NCORES = 8
B, S, D, F, O = 4, 2048, 1024, 4096, 1024
E, NS = 4, 2
T = B * S                 # 8192 tokens
P = 128
DK = D // P               # 8 k-tiles over D
DK2 = DK // 2             # 4 d k-tile pairs
FCH = F // P              # 32 f-chunks
FK2 = FCH // 2            # 16 f k-tile pairs

_CACHED = {}


def _build(blks, b1nz=False):
    """blks: tuple of per-block token counts, e.g. (256,)*16 + (128,)."""
    f32 = mybir.dt.float32
    bf = mybir.dt.bfloat16
    f8 = mybir.dt.float8e4
    AF = mybir.ActivationFunctionType
    ALU = mybir.AluOpType
    DR = mybir.MatmulPerfMode.DoubleRow
    NBLK = len(blks)
    BMAX = max(blks)

    nc = bacc.Bacc("TRN2", target_bir_lowering=False, debug=False)

    xq_d = nc.dram_tensor("xq", [NBLK, P, DK, BMAX], f8, kind="ExternalInput")
    xlo_d = nc.dram_tensor("xlo", [NBLK, P, DK, BMAX], f8, kind="ExternalInput")
    w1_d = nc.dram_tensor("w1", [P, 2 * DK2, 2 * FCH, P], f8,
                          kind="ExternalInput")
    w2_d = nc.dram_tensor("w2", [P, 2 * FCH, O], f8, kind="ExternalInput")
    b1_d = nc.dram_tensor("b1", [P, FCH], f32, kind="ExternalInput")  # 16*b1
    gw_d = nc.dram_tensor("gw", [P, 2 * DK, P], f8, kind="ExternalInput")
    gb_d = nc.dram_tensor("gb", [E, 1], f32, kind="ExternalInput")
    sel_d = nc.dram_tensor("sel", [E, 2], f32, kind="ExternalInput")
    yg_d = nc.dram_tensor("yg", [NBLK, BMAX, O], bf, kind="ExternalOutput")

    with tile.TileContext(nc) as tc:
        with (
            tc.tile_pool(name="wres", bufs=1) as wres,
            tc.tile_pool(name="consts", bufs=1) as consts,
            tc.tile_pool(name="xp", bufs=3) as xp,
            tc.tile_pool(name="xlp", bufs=3) as xlp,
            tc.tile_pool(name="hqp", bufs=2) as hqp,
            tc.tile_pool(name="hlp", bufs=2) as hlp,
            tc.tile_pool(name="hfp", bufs=3) as hfp,
            tc.tile_pool(name="gep", bufs=2) as gep,
            tc.tile_pool(name="sp", bufs=8) as sp,
            tc.tile_pool(name="outp", bufs=3) as outp,
            tc.tile_pool(name="l1ps", bufs=3, space="PSUM") as l1ps,
            tc.tile_pool(name="l2ps", bufs=3, space="PSUM") as l2ps,
            tc.tile_pool(name="gps", bufs=2, space="PSUM") as gps,
        ):
            # ---- consts first (tiny), then x(0), w1, x(1), w2: this DMA
            # issue order keeps the PE fed from ~2us.
            gwt = consts.tile([P, 2 * DK, P], f8, tag="gwt", name="gwt")
            nc.sync.dma_start(gwt[:], gw_d[:, :, :])
            gbt = consts.tile([E, 1], f32, tag="gbt", name="gbt")
            nc.sync.dma_start(gbt[:], gb_d[:, :])
            selt = consts.tile([E, 2], f32, tag="selt", name="selt")
            nc.sync.dma_start(selt[:], sel_d[:, :])
            if b1nz:
                b1t = consts.tile([P, FCH], f32, tag="b1t", name="b1t")
                nc.sync.dma_start(b1t[:], b1_d[:, :])

            hq_t = {}
            hlo_t = {}
            s_t = {}
            blk_x = {}

            def fetch_x(blk):
                Tb = blks[blk]
                xqt = xp.tile([P, DK, BMAX], f8, tag="xqt", name=f"xqt_{blk}")
                nc.sync.dma_start(xqt[:, :, :Tb], xq_d[blk, :, :, :Tb])
                xlt = xlp.tile([P, DK, BMAX], f8, tag="xlt", name=f"xlt_{blk}")
                nc.sync.dma_start(xlt[:, :, :Tb], xlo_d[blk, :, :, :Tb])
                blk_x[blk] = (xqt, xlt)

            fetch_x(0)
            w1t = wres.tile([P, 2 * DK2, 2 * FCH, P], f8, tag="w1t",
                            name="w1t")
            for c in range(16):
                csl = slice(c * 4, (c + 1) * 4)
                nc.sync.dma_start(w1t[:, :, csl, :], w1_d[:, :, csl, :])
            if NBLK > 1:
                fetch_x(1)
            w2t = wres.tile([P, 2 * FCH, O], f8, tag="w2t", name="w2t")
            for c in range(8):
                nc.sync.dma_start(w2t[:, c * 8:(c + 1) * 8, :],
                                  w2_d[:, c * 8:(c + 1) * 8, :])

            def emit_l2(blk):
                Tb = blks[blk]
                hq, hlo = hq_t[blk], hlo_t[blk]
                for tcg in range(Tb // P):
                    outt = outp.tile([P, O], bf, tag="outt",
                                     name=f"outt_{blk}_{tcg}")
                    for oh in range(2):
                        yp = l2ps.tile([P, 512], f32, tag="yp",
                                       name=f"yp_{blk}_{tcg}_{oh}")
                        osl = slice(oh * 512, (oh + 1) * 512)
                        for jp in range(FK2):
                            nc.tensor.matmul(
                                yp[:], lhsT=hq[:, 2 * jp + tcg, :, :],
                                rhs=w2t[:, 2 * jp:2 * jp + 2, osl],
                                perf_mode=DR, start=(jp == 0), stop=False)
                        for jp in range(FK2):
                            nc.tensor.matmul(
                                yp[:], lhsT=hlo[:, 2 * jp + tcg, :, :],
                                rhs=w2t[:, 2 * jp:2 * jp + 2, osl],
                                perf_mode=DR, start=False, stop=False)
                        for jp in range(FK2):
                            nc.tensor.matmul(
                                yp[:], lhsT=hq[:, 2 * jp + tcg, :, :],
                                rhs=w2t[:, 2 * FCH - 2 * FK2 + 2 * jp:
                                        2 * FCH - 2 * FK2 + 2 * jp + 2, osl],
                                perf_mode=DR, start=False, stop=(jp == FK2 - 1))
                        nc.vector.tensor_scalar(
                            outt[:, osl], yp[:], s_t[(blk, tcg)][:, 0:1], None,
                            ALU.mult)
                    nc.sync.dma_start(yg_d[blk, tcg * P:(tcg + 1) * P, :],
                                      outt[:])

            for blk in range(NBLK):
                Tb = blks[blk]
                ntc = Tb // P
                if blk not in blk_x:
                    fetch_x(blk)
                xqt, xlt = blk_x[blk]

                # ---- gating: logits^T [E(+pad), Tb] at scale 256 ----
                gpsu = gps.tile([P, BMAX], f32, tag="gpsu", name=f"gpsu_{blk}",
                                bufs=1)
                for j in range(DK2):
                    nc.tensor.matmul(
                        gpsu[:, :Tb], lhsT=gwt[:, 2 * j:2 * j + 2, :],
                        rhs=xqt[:, 2 * j:2 * j + 2, :Tb],
                        perf_mode=DR, start=(j == 0), stop=False)
                for j in range(DK2):
                    nc.tensor.matmul(
                        gpsu[:, :Tb], lhsT=gwt[:, 2 * j:2 * j + 2, :],
                        rhs=xlt[:, 2 * j:2 * j + 2, :Tb],
                        perf_mode=DR, start=False, stop=False)
                for j in range(DK2):
                    nc.tensor.matmul(
                        gpsu[:, :Tb], lhsT=gwt[:, DK + 2 * j:DK + 2 * j + 2, :],
                        rhs=xqt[:, 2 * j:2 * j + 2, :Tb],
                        perf_mode=DR, start=False, stop=(j == DK2 - 1))
                gext = gep.tile([E, BMAX], f32, tag="gext", name=f"gext_{blk}")
                nc.scalar.activation(gext[:, :Tb], gpsu[:E, :Tb], AF.Exp,
                                     bias=gbt[:, 0:1], scale=1.0 / 256.0)
                ndt = gps.tile([P, 2, 2], f32, tag="ndt", name=f"ndt_{blk}",
                               bufs=1)
                for tcg in range(ntc):
                    nc.tensor.matmul(
                        ndt[:, tcg, :], lhsT=gext[:, tcg * P:(tcg + 1) * P],
                        rhs=selt[:], start=True, stop=True)
                    rt = sp.tile([P, 1], f32, tag="rt", name=f"rt_{blk}_{tcg}")
                    nc.vector.reciprocal(rt[:], ndt[:, tcg, 1:2])
                    st = sp.tile([P, 1], f32, tag="st", name=f"st_{blk}_{tcg}")
                    nc.vector.tensor_tensor(st[:], ndt[:, tcg, 0:1], rt[:],
                                            ALU.mult)
                    s_t[(blk, tcg)] = st

                # ---- L1: h for all of F, pair-packed for L2 ldweights ----
                # hq/hlo [128, 32(=fpair*2+tokchunk), 2(f ktile), 128(tok)]
                hq = hqp.tile([P, FCH, 2, P], f8, tag="hq", name=f"hq_{blk}")
                hlo = hlp.tile([P, FCH, 2, P], f8, tag="hlo", name=f"hlo_{blk}")
                hq_t[blk], hlo_t[blk] = hq, hlo
                for fcp in range(FCH // 2):
                    ps = l1ps.tile([P, 2, 2, P], f32, tag="l1t",
                                   name=f"l1t_{blk}_{fcp}")
                    for half in range(2):
                        fc = 2 * fcp + half
                        pso = ps[:, half, :, :] if ntc == 2 else ps[:, half, 0, :]
                        for j in range(DK2):
                            nc.tensor.matmul(
                                pso, lhsT=w1t[:, j, 2 * fc:2 * fc + 2, :],
                                rhs=xqt[:, 2 * j:2 * j + 2, :Tb],
                                perf_mode=DR, start=(j == 0), stop=False)
                        for j in range(DK2):
                            nc.tensor.matmul(
                                pso, lhsT=w1t[:, j, 2 * fc:2 * fc + 2, :],
                                rhs=xlt[:, 2 * j:2 * j + 2, :Tb],
                                perf_mode=DR, start=False, stop=False)
                        for j in range(DK2):
                            nc.tensor.matmul(
                                pso,
                                lhsT=w1t[:, DK2 + j, 2 * fc:2 * fc + 2, :],
                                rhs=xqt[:, 2 * j:2 * j + 2, :Tb],
                                perf_mode=DR, start=False,
                                stop=(j == DK2 - 1))
                    hf = hfp.tile([P, 2, 2, P], bf, tag="hf",
                                  name=f"hf_{blk}_{fcp}")
                    # hf = relu(psum*2^-10 + b1)*16 == relu(psum*2^-6 + 16*b1)
                    if b1nz:
                        for half in range(2):
                            fc = 2 * fcp + half
                            nc.scalar.activation(
                                hf[:, half, :ntc, :], ps[:, half, :ntc, :],
                                AF.Relu, bias=b1t[:, fc:fc + 1],
                                scale=1.0 / 64.0)
                    else:
                        nc.scalar.activation(hf[:, :, :ntc, :],
                                             ps[:, :, :ntc, :], AF.Relu,
                                             bias=0.0, scale=1.0 / 64.0)
                    for half in range(2):
                        dsl = (slice(None), slice(2 * fcp, 2 * fcp + ntc),
                               half, slice(None))
                        if half == 0:
                            nc.scalar.activation(hq[dsl], hf[:, half, :ntc, :],
                                                 AF.Copy, bias=0.0, scale=1.0)
                        else:
                            nc.vector.tensor_scalar_mul(
                                hq[dsl], hf[:, half, :ntc, :], 1.0)
                        nc.vector.tensor_tensor(hlo[dsl], hf[:, half, :ntc, :],
                                                hq[dsl], ALU.subtract)

                if blk > 0:
                    emit_l2(blk - 1)
                    del hq_t[blk - 1], hlo_t[blk - 1], blk_x[blk - 1]
            emit_l2(NBLK - 1)

    nc.finalize()
    return nc


def _get_nc(blks, b1nz=False):
    key = (tuple(blks), b1nz)
    if key not in _CACHED:
        _CACHED[key] = _build(tuple(blks), b1nz)
    return _CACHED[key]


def _quant_pair(a, scale):
    """(base, residual) e4m3 value arrays for a*scale, residual at same scale."""
    hi = (a * scale).astype(E4)
    lo = (a * scale - hi.astype(np.float32)).astype(E4)
    return hi, lo


def kernel(**inputs) -> np.ndarray:
    x = np.asarray(inputs['x'], np.float32).reshape(T, D)
    gw = np.asarray(inputs['gate_w'], np.float32)
    gb = np.asarray(inputs['gate_b'], np.float32)
    sw1 = np.asarray(inputs['sw1'], np.float32)
    sb1 = np.asarray(inputs['sb1'], np.float32)
    sw2 = np.asarray(inputs['sw2'], np.float32)
    sb2 = np.asarray(inputs['sb2'], np.float32)
    rw1 = np.asarray(inputs['rw1'], np.float32)
    rb1 = np.asarray(inputs['rb1'], np.float32)
    rw2 = np.asarray(inputs['rw2'], np.float32)
    rb2 = np.asarray(inputs['rb2'], np.float32)
    assert not (np.any(sb2) or np.any(rb2)), "nonzero second-layer bias unsupported"

    # host gating (same fp32 math as reference) -> top-2 masks
    logits = x @ gw + gb
    m1 = logits.max(1, keepdims=True)
    pm = logits + (logits >= m1) * np.float32(-1e30)
    keep = logits >= pm.max(1, keepdims=True)
    counts = keep.sum(0)

    half = T // 2
    cap_needed = max(half, int(counts.max()))
    nfull, rem = divmod(cap_needed, 256)
    if rem == 0:
        blks = (256,) * nfull
    elif rem <= 128:
        blks = (256,) * nfull + (128,)
    else:
        blks = (256,) * (nfull + 1)
    cap = sum(blks)
    NBLK, BMAX = len(blks), max(blks)

    xq8, xlo8 = _quant_pair(x, np.float32(16.0))

    w1_all = np.concatenate([sw1, rw1], axis=0)
    w2_all = np.concatenate([sw2, rw2], axis=0)
    b1_all = np.concatenate([sb1, rb1], axis=0)
    w1p, w2p, b1p = [], [], []
    for e in range(NS + E):
        h1, l1 = _quant_pair(w1_all[e], np.float32(64.0))
        # [128, dpair(4 base + 4 resid), fchunk*2(ktile-in-pair), 128]
        packs = []
        for arr in (h1, l1):
            a5 = arr.reshape(DK2, 2, P, FCH, P)        # [jp, i, p, fc, c]
            packs.append(a5.transpose(2, 0, 3, 1, 4).reshape(P, DK2, 2 * FCH, P))
        w1p.append(np.ascontiguousarray(np.concatenate(packs, axis=1)))
        h2, l2 = _quant_pair(w2_all[e], np.float32(64.0))
        w2cat = np.concatenate([
            h2.reshape(FCH, P, O), l2.reshape(FCH, P, O)], axis=0)
        w2p.append(np.ascontiguousarray(w2cat.transpose(1, 0, 2)))
        b1p.append(np.ascontiguousarray(
            (16.0 * b1_all[e]).reshape(FCH, P).T.astype(np.float32)))
    gwh, gwl = _quant_pair(gw, np.float32(16.0))
    gwpad = np.zeros((2 * DK, P, P), E4)               # [ktile, p, col(pad 128)]
    gwpad[:DK, :, :E] = gwh.reshape(DK, P, E)
    gwpad[DK:, :, :E] = gwl.reshape(DK, P, E)
    gwp = np.ascontiguousarray(gwpad.transpose(1, 0, 2))
    gbp = gb.reshape(E, 1).astype(np.float32)

    core_expert = [0, 0, 1, 1, 2, 3, 4, 5]
    core_tokens = [
        np.arange(0, half), np.arange(half, T),
        np.arange(0, half), np.arange(half, T),
    ] + [np.nonzero(keep[:, r])[0] for r in range(E)]

    def xpack(tl, src):
        arr = np.zeros((cap, D), E4)
        arr[:len(tl)] = src[tl]
        out = np.zeros((NBLK, P, DK, BMAX), E4)
        t0 = 0
        for i, tb in enumerate(blks):
            blkarr = arr[t0:t0 + tb].astype(np.float32)  # [tb, D]
            out[i, :, :, :tb] = blkarr.reshape(tb, DK, P).transpose(2, 1, 0)
            t0 += tb
        return out

    in_maps = []
    for c in range(NCORES):
        e = core_expert[c]
        tl = core_tokens[c]
        assert len(tl) <= cap
        sel = np.zeros((E, 2), np.float32)
        sel[:, 1] = 1.0
        if c < 4:
            sel[:, 0] = 0.5 * (2.0 ** -10)
        else:
            sel[c - 4, 0] = 2.0 ** -10
        in_maps.append({
            "xq": xpack(tl, xq8), "xlo": xpack(tl, xlo8),
            "w1": w1p[e], "w2": w2p[e], "b1": b1p[e],
            "gw": gwp, "gb": gbp, "sel": sel,
        })

    b1nz = bool(np.any(sb1) or np.any(rb1))
    nc = _get_nc(blks, b1nz)
    res = run_bass_kernel_spmd(nc, in_maps, list(range(NCORES)))

    out = np.zeros((T, O), np.float32)
    for c in range(NCORES):
        tl = core_tokens[c]
        yg = np.asarray(res.results[c]["yg"]).reshape(NBLK, BMAX, O)
        yg = np.concatenate([yg[i, :tb] for i, tb in enumerate(blks)], axis=0)
        out[tl] += yg[:len(tl)].astype(np.float32)
        # general-case second-layer bias would be added here (zero for this net)
    return out.reshape(B, S, O)


# revision 13
# speedup vs baseline: 1.0252x; 1.0001x over previous
"""Trainium2 Bass kernel for a small MoE layer (4 routed top-2 + 2 shared experts).

Strategy: UNIFIED EXPERT-PARALLEL over 8 NeuronCores, all matmuls in fp8-e4m3
DoubleRow perf mode (256-deep contraction, 0.5 cycles/row -> 4x bf16 matmul
throughput in the TRN2 cost model).

Every (token, expert) pair becomes a unit of work: shared experts are "routed
with scale 0.5" for every token; routed experts use the host-computed top-2
selection (same fp32 math as the reference).  Totals: 2*8192 shared pairs +
16384 routed pairs = 32768.  With this seed the per-expert counts are
[4157, 4137, 4013, 4077] routed + 4096-token halves of the shared experts, so
EVERY CORE SERVES EXACTLY ONE EXPERT with capacity 4224 tokens (16 blocks of
256 + 1 of 128):
  core 0/1: shared expert 0 (tokens 0..4095 / 4096..8191)
  core 2/3: shared expert 1
  core 4+r: routed expert r (its selected tokens, host-gathered)

Numerics (rel err ~2e-3 vs fp32 reference, gate is 2e-2): e4m3 alone is too
coarse (each of x/w1/h/w2 alone gives ~2.4e-2), so every operand is split
base+residual at the SAME power-of-2 scale (residual relies on e4m3
subnormals) and the kernel runs 6 fp8 passes:
  L1: xq@w1q + xlo@w1q + xq@w1l          (pairs of 128-deep k-tiles)
  L2: hq@w2q + hlo@w2q + hq@w2l
  h  = relu(psum)*2^-6 in bf16 (ACT), hq = e4m3(h) (ACT), hlo = h - hq (DVE)
Gating runs on-device in transposed [E, tok] layout from the same fp8 x
(+ residual passes), exp on ACT, then a tiny f32 matmul against per-core
selector columns [sel*2^-10, ones] gives num/den per token; DVE reciprocal +
multiply yields the per-token drain scale (0.5*2^-10 for shared cores, softmax
prob * 2^-10 for routed).  The L2 drain is a DVE tensor_scalar multiply by
that [128,1] scale (psum carries 2^10 scaling), output bf16, host scatter-adds.

HW constraint (walrus 's3_lw_dual_fp8_restrictions'): a DoubleRow Ldweights
needs its two fp8 k-tiles CONTIGUOUS per partition with 128 active columns, so
every stationary operand is laid out pair-packed:
  w1 [128, 8(dpair: 4 base + 4 resid), 64(fchunk x2 ktile), 128]
  hq/hlo [128, 32(fpair x2 tokchunk), 2(ktile), 128]
  gw padded to 128 columns (experts in cols 0..3)
The moving operand (x, w2) has no such restriction.

Scales: x*16, w*64 -> L1 psum = 1024*x@w1; h stored as 16*relu(.) so L2 psum =
1024*h@w2; all residuals quantized at the base tensor's scale so every pass
accumulates into the same PSUM scale.
"""

import sys

sys.path.insert(0, '/opt/trn_rl_repo')

import numpy as np
import ml_dtypes

import concourse.bass as bass
import concourse.mybir as mybir
import concourse.tile as tile
from concourse import bacc
from concourse.bass_utils import run_bass_kernel_spmd

E4 = ml_dtypes.float8_e4m3
BF16 = ml_dtypes.bfloat16

NCORES = 8
B, S, D, F, O = 4, 2048, 1024, 4096, 1024
E, NS = 4, 2
T = B * S                 # 8192 tokens
P = 128
DK = D // P               # 8 k-tiles over D
DK2 = DK // 2             # 4 d k-tile pairs
FCH = F // P              # 32 f-chunks
FK2 = FCH // 2            # 16 f k-tile pairs

_CACHED = {}


def _build(blks, b1nz=False):
    """blks: tuple of per-block token counts, e.g. (256,)*16 + (128,)."""
    f32 = mybir.dt.float32
    bf = mybir.dt.bfloat16
    f8 = mybir.dt.float8e4
    AF = mybir.ActivationFunctionType
    ALU = mybir.AluOpType
    DR = mybir.MatmulPerfMode.DoubleRow
    NBLK = len(blks)
    BMAX = max(blks)

    nc = bacc.Bacc("TRN2", target_bir_lowering=False, debug=False)

    xq_d = nc.dram_tensor("xq", [NBLK, P, DK, BMAX], f8, kind="ExternalInput")
    xlo_d = nc.dram_tensor("xlo", [NBLK, P, DK, BMAX], f8, kind="ExternalInput")
    w1_d = nc.dram_tensor("w1", [P, 2 * DK2, 2 * FCH, P], f8,
                          kind="ExternalInput")
    w2_d = nc.dram_tensor("w2", [P, 2 * FCH, O], f8, kind="ExternalInput")
    b1_d = nc.dram_tensor("b1", [P, FCH], f32, kind="ExternalInput")  # 16*b1
    gw_d = nc.dram_tensor("gw", [P, 2 * DK, P], f8, kind="ExternalInput")
    gb_d = nc.dram_tensor("gb", [E, 1], f32, kind="ExternalInput")
    sel_d = nc.dram_tensor("sel", [E, 2], f32, kind="ExternalInput")
    yg_d = nc.dram_tensor("yg", [NBLK, BMAX, O], bf, kind="ExternalOutput")

    with tile.TileContext(nc) as tc:
        with (
            tc.tile_pool(name="wres", bufs=1) as wres,
            tc.tile_pool(name="consts", bufs=1) as consts,
            tc.tile_pool(name="xp", bufs=3) as xp,
            tc.tile_pool(name="xlp", bufs=3) as xlp,
            tc.tile_pool(name="hqp", bufs=2) as hqp,
            tc.tile_pool(name="hlp", bufs=2) as hlp,
            tc.tile_pool(name="hfp", bufs=6) as hfp,
            tc.tile_pool(name="gep", bufs=2) as gep,
            tc.tile_pool(name="sp", bufs=8) as sp,
            tc.tile_pool(name="outp", bufs=3) as outp,
            tc.tile_pool(name="l1ps", bufs=3, space="PSUM") as l1ps,
            tc.tile_pool(name="l2ps", bufs=3, space="PSUM") as l2ps,
            tc.tile_pool(name="gps", bufs=2, space="PSUM") as gps,
        ):
            # ---- consts first (tiny), then x(0), w1, x(1), w2: this DMA
            # issue order keeps the PE fed from ~2us.
            gwt = consts.tile([P, 2 * DK, P], f8, tag="gwt", name="gwt")
            nc.sync.dma_start(gwt[:], gw_d[:, :, :])
            gbt = consts.tile([E, 1], f32, tag="gbt", name="gbt")
            nc.sync.dma_start(gbt[:], gb_d[:, :])
            selt = consts.tile([E, 2], f32, tag="selt", name="selt")
            nc.sync.dma_start(selt[:], sel_d[:, :])
            if b1nz:
                b1t = consts.tile([P, FCH], f32, tag="b1t", name="b1t")
                nc.sync.dma_start(b1t[:], b1_d[:, :])

            hq_t = {}
            hlo_t = {}
            s_t = {}
            blk_x = {}

            def fetch_x(blk):
                Tb = blks[blk]
                xqt = xp.tile([P, DK, BMAX], f8, tag="xqt", name=f"xqt_{blk}")
                nc.sync.dma_start(xqt[:, :, :Tb], xq_d[blk, :, :, :Tb])
                xlt = xlp.tile([P, DK, BMAX], f8, tag="xlt", name=f"xlt_{blk}")
                nc.sync.dma_start(xlt[:, :, :Tb], xlo_d[blk, :, :, :Tb])
                blk_x[blk] = (xqt, xlt)

            fetch_x(0)
            w1t = wres.tile([P, 2 * DK2, 2 * FCH, P], f8, tag="w1t",
                            name="w1t")
            for c in range(16):
                csl = slice(c * 4, (c + 1) * 4)
                nc.sync.dma_start(w1t[:, :, csl, :], w1_d[:, :, csl, :])
            if NBLK > 1:
                fetch_x(1)
            w2t = wres.tile([P, 2 * FCH, O], f8, tag="w2t", name="w2t")
            for c in range(8):
                nc.sync.dma_start(w2t[:, c * 8:(c + 1) * 8, :],
                                  w2_d[:, c * 8:(c + 1) * 8, :])

            def emit_l2(blk):
                Tb = blks[blk]
                hq, hlo = hq_t[blk], hlo_t[blk]
                for tcg in range(Tb // P):
                    outt = outp.tile([P, O], bf, tag="outt",
                                     name=f"outt_{blk}_{tcg}")
                    for oh in range(2):
                        yp = l2ps.tile([P, 512], f32, tag="yp",
                                       name=f"yp_{blk}_{tcg}_{oh}")
                        osl = slice(oh * 512, (oh + 1) * 512)
                        for jp in range(FK2):
                            nc.tensor.matmul(
                                yp[:], lhsT=hq[:, 2 * jp + tcg, :, :],
                                rhs=w2t[:, 2 * jp:2 * jp + 2, osl],
                                perf_mode=DR, start=(jp == 0), stop=False)
                        for jp in range(FK2):
                            nc.tensor.matmul(
                                yp[:], lhsT=hlo[:, 2 * jp + tcg, :, :],
                                rhs=w2t[:, 2 * jp:2 * jp + 2, osl],
                                perf_mode=DR, start=False, stop=False)
                        for jp in range(FK2):
                            nc.tensor.matmul(
                                yp[:], lhsT=hq[:, 2 * jp + tcg, :, :],
                                rhs=w2t[:, 2 * FCH - 2 * FK2 + 2 * jp:
                                        2 * FCH - 2 * FK2 + 2 * jp + 2, osl],
                                perf_mode=DR, start=False, stop=(jp == FK2 - 1))
                        nc.vector.tensor_scalar(
                            outt[:, osl], yp[:], s_t[(blk, tcg)][:, 0:1], None,
                            ALU.mult)
                    nc.sync.dma_start(yg_d[blk, tcg * P:(tcg + 1) * P, :],
                                      outt[:])

            for blk in range(NBLK):
                Tb = blks[blk]
                ntc = Tb // P
                if blk not in blk_x:
                    fetch_x(blk)
                xqt, xlt = blk_x[blk]

                # ---- gating: logits^T [E(+pad), Tb] at scale 256 ----
                gpsu = gps.tile([P, BMAX], f32, tag="gpsu", name=f"gpsu_{blk}",
                                bufs=1)
                for j in range(DK2):
                    nc.tensor.matmul(
                        gpsu[:, :Tb], lhsT=gwt[:, 2 * j:2 * j + 2, :],
                        rhs=xqt[:, 2 * j:2 * j + 2, :Tb],
                        perf_mode=DR, start=(j == 0), stop=False)
                for j in range(DK2):
                    nc.tensor.matmul(
                        gpsu[:, :Tb], lhsT=gwt[:, 2 * j:2 * j + 2, :],
                        rhs=xlt[:, 2 * j:2 * j + 2, :Tb],
                        perf_mode=DR, start=False, stop=False)
                for j in range(DK2):
                    nc.tensor.matmul(
                        gpsu[:, :Tb], lhsT=gwt[:, DK + 2 * j:DK + 2 * j + 2, :],
                        rhs=xqt[:, 2 * j:2 * j + 2, :Tb],
                        perf_mode=DR, start=False, stop=(j == DK2 - 1))
                gext = gep.tile([E, BMAX], f32, tag="gext", name=f"gext_{blk}")
                nc.scalar.activation(gext[:, :Tb], gpsu[:E, :Tb], AF.Exp,
                                     bias=gbt[:, 0:1], scale=1.0 / 256.0)
                ndt = gps.tile([P, 2, 2], f32, tag="ndt", name=f"ndt_{blk}",
                               bufs=1)
                for tcg in range(ntc):
                    nc.tensor.matmul(
                        ndt[:, tcg, :], lhsT=gext[:, tcg * P:(tcg + 1) * P],
                        rhs=selt[:], start=True, stop=True)
                    rt = sp.tile([P, 1], f32, tag="rt", name=f"rt_{blk}_{tcg}")
                    nc.vector.reciprocal(rt[:], ndt[:, tcg, 1:2])
                    st = sp.tile([P, 1], f32, tag="st", name=f"st_{blk}_{tcg}")
                    nc.vector.tensor_tensor(st[:], ndt[:, tcg, 0:1], rt[:],
                                            ALU.mult)
                    s_t[(blk, tcg)] = st

                # ---- L1: h for all of F, pair-packed for L2 ldweights ----
                # hq/hlo [128, 32(=fpair*2+tokchunk), 2(f ktile), 128(tok)]
                hq = hqp.tile([P, FCH, 2, P], f8, tag="hq", name=f"hq_{blk}")
                hlo = hlp.tile([P, FCH, 2, P], f8, tag="hlo", name=f"hlo_{blk}")
                hq_t[blk], hlo_t[blk] = hq, hlo
                for fcp in range(FCH // 2):
                    ps = l1ps.tile([P, 2, 2, P], f32, tag="l1t",
                                   name=f"l1t_{blk}_{fcp}")
                    for half in range(2):
                        fc = 2 * fcp + half
                        pso = ps[:, half, :, :] if ntc == 2 else ps[:, half, 0, :]
                        for j in range(DK2):
                            nc.tensor.matmul(
                                pso, lhsT=w1t[:, j, 2 * fc:2 * fc + 2, :],
                                rhs=xqt[:, 2 * j:2 * j + 2, :Tb],
                                perf_mode=DR, start=(j == 0), stop=False)
                        for j in range(DK2):
                            nc.tensor.matmul(
                                pso, lhsT=w1t[:, j, 2 * fc:2 * fc + 2, :],
                                rhs=xlt[:, 2 * j:2 * j + 2, :Tb],
                                perf_mode=DR, start=False, stop=False)
                        for j in range(DK2):
                            nc.tensor.matmul(
                                pso,
                                lhsT=w1t[:, DK2 + j, 2 * fc:2 * fc + 2, :],
                                rhs=xqt[:, 2 * j:2 * j + 2, :Tb],
                                perf_mode=DR, start=False,
                                stop=(j == DK2 - 1))
                    hf = hfp.tile([P, 2, 2, P], bf, tag="hf",
                                  name=f"hf_{blk}_{fcp}")
                    # hf = relu(psum*2^-10 + b1)*16 == relu(psum*2^-6 + 16*b1)
                    if b1nz:
                        for half in range(2):
                            fc = 2 * fcp + half
                            nc.scalar.activation(
                                hf[:, half, :ntc, :], ps[:, half, :ntc, :],
                                AF.Relu, bias=b1t[:, fc:fc + 1],
                                scale=1.0 / 64.0)
                    else:
                        nc.scalar.activation(hf[:, :, :ntc, :],
                                             ps[:, :, :ntc, :], AF.Relu,
                                             bias=0.0, scale=1.0 / 64.0)
                    for half in range(2):
                        dsl = (slice(None), slice(2 * fcp, 2 * fcp + ntc),
                               half, slice(None))
                        if half == 0:
                            nc.scalar.activation(hq[dsl], hf[:, half, :ntc, :],
                                                 AF.Copy, bias=0.0, scale=1.0)
                        else:
                            nc.vector.tensor_scalar_mul(
                                hq[dsl], hf[:, half, :ntc, :], 1.0)
                        nc.vector.tensor_tensor(hlo[dsl], hf[:, half, :ntc, :],
                                                hq[dsl], ALU.subtract)

                if blk > 0:
                    emit_l2(blk - 1)
                    del hq_t[blk - 1], hlo_t[blk - 1], blk_x[blk - 1]
            emit_l2(NBLK - 1)

    nc.finalize()
    return nc


def _get_nc(blks, b1nz=False):
    key = (tuple(blks), b1nz)
    if key not in _CACHED:
        _CACHED[key] = _build(tuple(blks), b1nz)
    return _CACHED[key]


def _quant_pair(a, scale):
    """(base, residual) e4m3 value arrays for a*scale, residual at same scale."""
    hi = (a * scale).astype(E4)
    lo = (a * scale - hi.astype(np.float32)).astype(E4)
    return hi, lo


def kernel(**inputs) -> np.ndarray:
    x = np.asarray(inputs['x'], np.float32).reshape(T, D)
    gw = np.asarray(inputs['gate_w'], np.float32)
    gb = np.asarray(inputs['gate_b'], np.float32)
    sw1 = np.asarray(inputs['sw1'], np.float32)
    sb1 = np.asarray(inputs['sb1'], np.float32)
    sw2 = np.asarray(inputs['sw2'], np.float32)
    sb2 = np.asarray(inputs['sb2'], np.float32)
    rw1 = np.asarray(inputs['rw1'], np.float32)
    rb1 = np.asarray(inputs['rb1'], np.float32)
    rw2 = np.asarray(inputs['rw2'], np.float32)
    rb2 = np.asarray(inputs['rb2'], np.float32)
    assert not (np.any(sb2) or np.any(rb2)), "nonzero second-layer bias unsupported"

    # host gating (same fp32 math as reference) -> top-2 masks
    logits = x @ gw + gb
    m1 = logits.max(1, keepdims=True)
    pm = logits + (logits >= m1) * np.float32(-1e30)
    keep = logits >= pm.max(1, keepdims=True)
    counts = keep.sum(0)

    half = T // 2
    cap_needed = max(half, int(counts.max()))
    nfull, rem = divmod(cap_needed, 256)
    if rem == 0:
        blks = (256,) * nfull
    elif rem <= 128:
        blks = (256,) * nfull + (128,)
    else:
        blks = (256,) * (nfull + 1)
    cap = sum(blks)
    NBLK, BMAX = len(blks), max(blks)

    xq8, xlo8 = _quant_pair(x, np.float32(16.0))

    w1_all = np.concatenate([sw1, rw1], axis=0)
    w2_all = np.concatenate([sw2, rw2], axis=0)
    b1_all = np.concatenate([sb1, rb1], axis=0)
    w1p, w2p, b1p = [], [], []
    for e in range(NS + E):
        h1, l1 = _quant_pair(w1_all[e], np.float32(64.0))
        # [128, dpair(4 base + 4 resid), fchunk*2(ktile-in-pair), 128]
        packs = []
        for arr in (h1, l1):
            a5 = arr.reshape(DK2, 2, P, FCH, P)        # [jp, i, p, fc, c]
            packs.append(a5.transpose(2, 0, 3, 1, 4).reshape(P, DK2, 2 * FCH, P))
        w1p.append(np.ascontiguousarray(np.concatenate(packs, axis=1)))
        h2, l2 = _quant_pair(w2_all[e], np.float32(64.0))
        w2cat = np.concatenate([
            h2.reshape(FCH, P, O), l2.reshape(FCH, P, O)], axis=0)
        w2p.append(np.ascontiguousarray(w2cat.transpose(1, 0, 2)))
        b1p.append(np.ascontiguousarray(
            (16.0 * b1_all[e]).reshape(FCH, P).T.astype(np.float32)))
    gwh, gwl = _quant_pair(gw, np.float32(16.0))
    gwpad = np.zeros((2 * DK, P, P), E4)               # [ktile, p, col(pad 128)]
    gwpad[:DK, :, :E] = gwh.reshape(DK, P, E)
    gwpad[DK:, :, :E] = gwl.reshape(DK, P, E)
    gwp = np.ascontiguousarray(gwpad.transpose(1, 0, 2))
    gbp = gb.reshape(E, 1).astype(np.float32)

    core_expert = [0, 0, 1, 1, 2, 3, 4, 5]
    core_tokens = [
        np.arange(0, half), np.arange(half, T),
        np.arange(0, half), np.arange(half, T),
    ] + [np.nonzero(keep[:, r])[0] for r in range(E)]

    def xpack(tl, src):
        arr = np.zeros((cap, D), E4)
        arr[:len(tl)] = src[tl]
        out = np.zeros((NBLK, P, DK, BMAX), E4)
        t0 = 0
        for i, tb in enumerate(blks):
            blkarr = arr[t0:t0 + tb].astype(np.float32)  # [tb, D]
            out[i, :, :, :tb] = blkarr.reshape(tb, DK, P).transpose(2, 1, 0)
            t0 += tb
        return out

    in_maps = []
    for c in range(NCORES):
        e = core_expert[c]
        tl = core_tokens[c]
        assert len(tl) <= cap
        sel = np.zeros((E, 2), np.float32)
        sel[:, 1] = 1.0
        if c < 4:
            sel[:, 0] = 0.5 * (2.0 ** -10)
        else:
            sel[c - 4, 0] = 2.0 ** -10
        in_maps.append({
            "xq": xpack(tl, xq8), "xlo": xpack(tl, xlo8),
            "w1": w1p[e], "w2": w2p[e], "b1": b1p[e],
            "gw": gwp, "gb": gbp, "sel": sel,
        })

    b1nz = bool(np.any(sb1) or np.any(rb1))
    nc = _get_nc(blks, b1nz)
    res = run_bass_kernel_spmd(nc, in_maps, list(range(NCORES)))

    out = np.zeros((T, O), np.float32)
    for c in range(NCORES):
        tl = core_tokens[c]
        yg = np.asarray(res.results[c]["yg"]).reshape(NBLK, BMAX, O)
        yg = np.concatenate([yg[i, :tb] for i, tb in enumerate(blks)], axis=0)
        out[tl] += yg[:len(tl)].astype(np.float32)
        # general-case second-layer bias would be added here (zero for this net)
    return out.reshape(B, S, O)


# revision 14
# speedup vs baseline: 1.0668x; 1.0406x over previous
"""Trainium2 Bass kernel for a small MoE layer (4 routed top-2 + 2 shared experts).

Strategy: UNIFIED EXPERT-PARALLEL over 8 NeuronCores, all matmuls in fp8-e4m3
DoubleRow perf mode (256-deep contraction, 0.5 cycles/row -> 4x bf16 matmul
throughput in the TRN2 cost model).

Every (token, expert) pair becomes a unit of work: shared experts are "routed
with scale 0.5" for every token; routed experts use the host-computed top-2
selection (same fp32 math as the reference).  Totals: 2*8192 shared pairs +
16384 routed pairs = 32768.  With this seed the per-expert counts are
[4157, 4137, 4013, 4077] routed + 4096-token halves of the shared experts, so
EVERY CORE SERVES EXACTLY ONE EXPERT with capacity 4224 tokens (16 blocks of
256 + 1 of 128):
  core 0/1: shared expert 0 (tokens 0..4095 / 4096..8191)
  core 2/3: shared expert 1
  core 4+r: routed expert r (its selected tokens, host-gathered)

Numerics (rel err ~2e-3 vs fp32 reference, gate is 2e-2): e4m3 alone is too
coarse (each of x/w1/h/w2 alone gives ~2.4e-2), so every operand is split
base+residual at the SAME power-of-2 scale (residual relies on e4m3
subnormals) and the kernel runs 6 fp8 passes:
  L1: xq@w1q + xlo@w1q + xq@w1l          (pairs of 128-deep k-tiles)
  L2: hq@w2q + hlo@w2q + hq@w2l
  h  = relu(psum)*2^-6 in bf16 (ACT), hq = e4m3(h) (ACT), hlo = h - hq (DVE)
Gating runs on-device in transposed [E, tok] layout from the same fp8 x
(+ residual passes), exp on ACT, then a tiny f32 matmul against per-core
selector columns [sel*2^-10, ones] gives num/den per token; DVE reciprocal +
multiply yields the per-token drain scale (0.5*2^-10 for shared cores, softmax
prob * 2^-10 for routed).  The L2 drain is a DVE tensor_scalar multiply by
that [128,1] scale (psum carries 2^10 scaling), output bf16, host scatter-adds.

HW constraint (walrus 's3_lw_dual_fp8_restrictions'): a DoubleRow Ldweights
needs its two fp8 k-tiles CONTIGUOUS per partition with 128 active columns, so
every stationary operand is laid out pair-packed:
  w1 [128, 8(dpair: 4 base + 4 resid), 64(fchunk x2 ktile), 128]
  hq/hlo [128, 32(fpair x2 tokchunk), 2(ktile), 128]
  gw padded to 128 columns (experts in cols 0..3)
The moving operand (x, w2) has no such restriction.

Scales: x*16, w*64 -> L1 psum = 1024*x@w1; h stored as 16*relu(.) so L2 psum =
1024*h@w2; all residuals quantized at the base tensor's scale so every pass
accumulates into the same PSUM scale.
"""

import sys

sys.path.insert(0, '/opt/trn_rl_repo')

import numpy as np
import ml_dtypes

import concourse.bass as bass
import concourse.mybir as mybir
import concourse.tile as tile
from concourse import bacc
from concourse.bass_utils import run_bass_kernel_spmd

E4 = ml_dtypes.float8_e4m3
BF16 = ml_dtypes.bfloat16

NCORES = 8
B, S, D, F, O = 4, 2048, 1024, 4096, 1024
E, NS = 4, 2
T = B * S                 # 8192 tokens
P = 128
DK = D // P               # 8 k-tiles over D
DK2 = DK // 2             # 4 d k-tile pairs
FCH = F // P              # 32 f-chunks
FK2 = FCH // 2            # 16 f k-tile pairs

_CACHED = {}


def _build(blks, b1nz=False):
    """blks: tuple of per-block token counts, e.g. (256,)*16 + (128,)."""
    f32 = mybir.dt.float32
    bf = mybir.dt.bfloat16
    f8 = mybir.dt.float8e4
    AF = mybir.ActivationFunctionType
    ALU = mybir.AluOpType
    DR = mybir.MatmulPerfMode.DoubleRow
    NBLK = len(blks)
    BMAX = max(blks)

    nc = bacc.Bacc("TRN2", target_bir_lowering=False, debug=False)

    xq_d = nc.dram_tensor("xq", [NBLK, P, DK, BMAX], f8, kind="ExternalInput")
    xlo_d = nc.dram_tensor("xlo", [NBLK, P, DK, BMAX], f8, kind="ExternalInput")
    w1_d = nc.dram_tensor("w1", [P, 2 * DK2, 2 * FCH, P], f8,
                          kind="ExternalInput")
    w2_d = nc.dram_tensor("w2", [P, 2 * FCH, O], f8, kind="ExternalInput")
    b1_d = nc.dram_tensor("b1", [P, FCH], f32, kind="ExternalInput")  # 16*b1
    gw_d = nc.dram_tensor("gw", [P, 2 * DK, P], f8, kind="ExternalInput")
    gb_d = nc.dram_tensor("gb", [E, 1], f32, kind="ExternalInput")
    sel_d = nc.dram_tensor("sel", [E, 2], f32, kind="ExternalInput")
    yg_d = nc.dram_tensor("yg", [NBLK, BMAX, O], bf, kind="ExternalOutput")

    with tile.TileContext(nc) as tc:
        with (
            tc.tile_pool(name="wres", bufs=1) as wres,
            tc.tile_pool(name="consts", bufs=1) as consts,
            tc.tile_pool(name="xp", bufs=3) as xp,
            tc.tile_pool(name="xlp", bufs=3) as xlp,
            tc.tile_pool(name="hqp", bufs=2) as hqp,
            tc.tile_pool(name="hlp", bufs=2) as hlp,
            tc.tile_pool(name="hfp", bufs=6) as hfp,
            tc.tile_pool(name="gep", bufs=2) as gep,
            tc.tile_pool(name="sp", bufs=8) as sp,
            tc.tile_pool(name="outp", bufs=3) as outp,
            tc.tile_pool(name="l1ps", bufs=3, space="PSUM") as l1ps,
            tc.tile_pool(name="l2ps", bufs=3, space="PSUM") as l2ps,
            tc.tile_pool(name="gps", bufs=2, space="PSUM") as gps,
        ):
            # ---- consts first (tiny), then x(0), w1, x(1), w2: this DMA
            # issue order keeps the PE fed from ~2us.
            gwt = consts.tile([P, 2 * DK, P], f8, tag="gwt", name="gwt")
            nc.sync.dma_start(gwt[:], gw_d[:, :, :])
            gbt = consts.tile([E, 1], f32, tag="gbt", name="gbt")
            nc.sync.dma_start(gbt[:], gb_d[:, :])
            selt = consts.tile([E, 2], f32, tag="selt", name="selt")
            nc.sync.dma_start(selt[:], sel_d[:, :])
            if b1nz:
                b1t = consts.tile([P, FCH], f32, tag="b1t", name="b1t")
                nc.sync.dma_start(b1t[:], b1_d[:, :])

            hq_t = {}
            hlo_t = {}
            s_t = {}
            blk_x = {}

            def fetch_x(blk):
                Tb = blks[blk]
                xqt = xp.tile([P, DK, BMAX], f8, tag="xqt", name=f"xqt_{blk}")
                nc.sync.dma_start(xqt[:, :, :Tb], xq_d[blk, :, :, :Tb])
                xlt = xlp.tile([P, DK, BMAX], f8, tag="xlt", name=f"xlt_{blk}")
                nc.sync.dma_start(xlt[:, :, :Tb], xlo_d[blk, :, :, :Tb])
                blk_x[blk] = (xqt, xlt)

            fetch_x(0)
            w1t = wres.tile([P, 2 * DK2, 2 * FCH, P], f8, tag="w1t",
                            name="w1t")
            for c in range(16):
                csl = slice(c * 4, (c + 1) * 4)
                nc.sync.dma_start(w1t[:, :, csl, :], w1_d[:, :, csl, :])
            if NBLK > 1:
                fetch_x(1)
            w2t = wres.tile([P, 2 * FCH, O], f8, tag="w2t", name="w2t")
            for c in range(8):
                nc.sync.dma_start(w2t[:, c * 8:(c + 1) * 8, :],
                                  w2_d[:, c * 8:(c + 1) * 8, :])

            def emit_l2(blk):
                Tb = blks[blk]
                hq, hlo = hq_t[blk], hlo_t[blk]
                for tcg in range(Tb // P):
                    outt = outp.tile([P, O], bf, tag="outt",
                                     name=f"outt_{blk}_{tcg}")
                    for oh in range(2):
                        yp = l2ps.tile([P, 512], f32, tag="yp",
                                       name=f"yp_{blk}_{tcg}_{oh}")
                        osl = slice(oh * 512, (oh + 1) * 512)
                        ops = []
                        for jp in range(FK2):
                            ops.append((hq[:, 2 * jp + tcg, :, :],
                                        w2t[:, 2 * jp:2 * jp + 2, osl]))
                        for jp in range(FK2 - 1):      # trimmed residual pass
                            ops.append((hlo[:, 2 * jp + tcg, :, :],
                                        w2t[:, 2 * jp:2 * jp + 2, osl]))
                        for jp in range(FK2 - 1):      # trimmed residual pass
                            ops.append((hq[:, 2 * jp + tcg, :, :],
                                        w2t[:, FCH + 2 * jp:
                                            FCH + 2 * jp + 2, osl]))
                        for i, (lt, rt) in enumerate(ops):
                            nc.tensor.matmul(
                                yp[:], lhsT=lt, rhs=rt, perf_mode=DR,
                                start=(i == 0), stop=(i == len(ops) - 1))
                        nc.vector.tensor_scalar(
                            outt[:, osl], yp[:], s_t[(blk, tcg)][:, 0:1], None,
                            ALU.mult)
                    nc.sync.dma_start(yg_d[blk, tcg * P:(tcg + 1) * P, :],
                                      outt[:])

            for blk in range(NBLK):
                Tb = blks[blk]
                ntc = Tb // P
                if blk not in blk_x:
                    fetch_x(blk)
                xqt, xlt = blk_x[blk]

                # ---- gating: logits^T [E(+pad), Tb] at scale 256 ----
                gpsu = gps.tile([P, BMAX], f32, tag="gpsu", name=f"gpsu_{blk}",
                                bufs=1)
                for j in range(DK2):
                    nc.tensor.matmul(
                        gpsu[:, :Tb], lhsT=gwt[:, 2 * j:2 * j + 2, :],
                        rhs=xqt[:, 2 * j:2 * j + 2, :Tb],
                        perf_mode=DR, start=(j == 0), stop=False)
                for j in range(DK2):
                    nc.tensor.matmul(
                        gpsu[:, :Tb], lhsT=gwt[:, 2 * j:2 * j + 2, :],
                        rhs=xlt[:, 2 * j:2 * j + 2, :Tb],
                        perf_mode=DR, start=False, stop=False)
                for j in range(DK2):
                    nc.tensor.matmul(
                        gpsu[:, :Tb], lhsT=gwt[:, DK + 2 * j:DK + 2 * j + 2, :],
                        rhs=xqt[:, 2 * j:2 * j + 2, :Tb],
                        perf_mode=DR, start=False, stop=(j == DK2 - 1))
                gext = gep.tile([E, BMAX], f32, tag="gext", name=f"gext_{blk}")
                nc.scalar.activation(gext[:, :Tb], gpsu[:E, :Tb], AF.Exp,
                                     bias=gbt[:, 0:1], scale=1.0 / 256.0)
                ndt = gps.tile([P, 2, 2], f32, tag="ndt", name=f"ndt_{blk}",
                               bufs=1)
                for tcg in range(ntc):
                    nc.tensor.matmul(
                        ndt[:, tcg, :], lhsT=gext[:, tcg * P:(tcg + 1) * P],
                        rhs=selt[:], start=True, stop=True)
                    rt = sp.tile([P, 1], f32, tag="rt", name=f"rt_{blk}_{tcg}")
                    nc.vector.reciprocal(rt[:], ndt[:, tcg, 1:2])
                    st = sp.tile([P, 1], f32, tag="st", name=f"st_{blk}_{tcg}")
                    nc.vector.tensor_tensor(st[:], ndt[:, tcg, 0:1], rt[:],
                                            ALU.mult)
                    s_t[(blk, tcg)] = st

                # ---- L1: h for all of F, pair-packed for L2 ldweights ----
                # hq/hlo [128, 32(=fpair*2+tokchunk), 2(f ktile), 128(tok)]
                hq = hqp.tile([P, FCH, 2, P], f8, tag="hq", name=f"hq_{blk}")
                hlo = hlp.tile([P, FCH, 2, P], f8, tag="hlo", name=f"hlo_{blk}")
                hq_t[blk], hlo_t[blk] = hq, hlo
                for fcp in range(FCH // 2):
                    ps = l1ps.tile([P, 2, 2, P], f32, tag="l1t",
                                   name=f"l1t_{blk}_{fcp}")
                    trim = (fcp == FCH // 2 - 1)
                    for half in range(2):
                        fc = 2 * fcp + half
                        pso = ps[:, half, :, :] if ntc == 2 else ps[:, half, 0, :]
                        ops = []
                        for j in range(DK2):
                            ops.append((w1t[:, j, 2 * fc:2 * fc + 2, :],
                                        xqt[:, 2 * j:2 * j + 2, :Tb]))
                        if not trim:   # residual passes, trimmed on last fcp
                            for j in range(DK2):
                                ops.append((w1t[:, j, 2 * fc:2 * fc + 2, :],
                                            xlt[:, 2 * j:2 * j + 2, :Tb]))
                            for j in range(DK2):
                                ops.append(
                                    (w1t[:, DK2 + j, 2 * fc:2 * fc + 2, :],
                                     xqt[:, 2 * j:2 * j + 2, :Tb]))
                        for i, (lt, rt) in enumerate(ops):
                            nc.tensor.matmul(
                                pso, lhsT=lt, rhs=rt, perf_mode=DR,
                                start=(i == 0), stop=(i == len(ops) - 1))
                    hf = hfp.tile([P, 2, 2, P], bf, tag="hf",
                                  name=f"hf_{blk}_{fcp}")
                    # hf = relu(psum*2^-10 + b1)*16 == relu(psum*2^-6 + 16*b1)
                    if b1nz:
                        for half in range(2):
                            fc = 2 * fcp + half
                            nc.scalar.activation(
                                hf[:, half, :ntc, :], ps[:, half, :ntc, :],
                                AF.Relu, bias=b1t[:, fc:fc + 1],
                                scale=1.0 / 64.0)
                    else:
                        nc.scalar.activation(hf[:, :, :ntc, :],
                                             ps[:, :, :ntc, :], AF.Relu,
                                             bias=0.0, scale=1.0 / 64.0)
                    for half in range(2):
                        dsl = (slice(None), slice(2 * fcp, 2 * fcp + ntc),
                               half, slice(None))
                        if half == 0:
                            nc.scalar.activation(hq[dsl], hf[:, half, :ntc, :],
                                                 AF.Copy, bias=0.0, scale=1.0)
                        else:
                            nc.vector.tensor_scalar_mul(
                                hq[dsl], hf[:, half, :ntc, :], 1.0)
                        if not trim:
                            nc.vector.tensor_tensor(
                                hlo[dsl], hf[:, half, :ntc, :], hq[dsl],
                                ALU.subtract)

                if blk > 0:
                    emit_l2(blk - 1)
                    del hq_t[blk - 1], hlo_t[blk - 1], blk_x[blk - 1]
            emit_l2(NBLK - 1)

    nc.finalize()
    return nc


def _get_nc(blks, b1nz=False):
    key = (tuple(blks), b1nz)
    if key not in _CACHED:
        _CACHED[key] = _build(tuple(blks), b1nz)
    return _CACHED[key]


def _quant_pair(a, scale):
    """(base, residual) e4m3 value arrays for a*scale, residual at same scale."""
    hi = (a * scale).astype(E4)
    lo = (a * scale - hi.astype(np.float32)).astype(E4)
    return hi, lo


def kernel(**inputs) -> np.ndarray:
    x = np.asarray(inputs['x'], np.float32).reshape(T, D)
    gw = np.asarray(inputs['gate_w'], np.float32)
    gb = np.asarray(inputs['gate_b'], np.float32)
    sw1 = np.asarray(inputs['sw1'], np.float32)
    sb1 = np.asarray(inputs['sb1'], np.float32)
    sw2 = np.asarray(inputs['sw2'], np.float32)
    sb2 = np.asarray(inputs['sb2'], np.float32)
    rw1 = np.asarray(inputs['rw1'], np.float32)
    rb1 = np.asarray(inputs['rb1'], np.float32)
    rw2 = np.asarray(inputs['rw2'], np.float32)
    rb2 = np.asarray(inputs['rb2'], np.float32)
    assert not (np.any(sb2) or np.any(rb2)), "nonzero second-layer bias unsupported"

    # host gating (same fp32 math as reference) -> top-2 masks
    logits = x @ gw + gb
    m1 = logits.max(1, keepdims=True)
    pm = logits + (logits >= m1) * np.float32(-1e30)
    keep = logits >= pm.max(1, keepdims=True)
    counts = keep.sum(0)

    half = T // 2
    cap_needed = max(half, int(counts.max()))
    nfull, rem = divmod(cap_needed, 256)
    if rem == 0:
        blks = (256,) * nfull
    elif rem <= 128:
        blks = (256,) * nfull + (128,)
    else:
        blks = (256,) * (nfull + 1)
    cap = sum(blks)
    NBLK, BMAX = len(blks), max(blks)

    xq8, xlo8 = _quant_pair(x, np.float32(16.0))

    w1_all = np.concatenate([sw1, rw1], axis=0)
    w2_all = np.concatenate([sw2, rw2], axis=0)
    b1_all = np.concatenate([sb1, rb1], axis=0)
    w1p, w2p, b1p = [], [], []
    for e in range(NS + E):
        h1, l1 = _quant_pair(w1_all[e], np.float32(64.0))
        # [128, dpair(4 base + 4 resid), fchunk*2(ktile-in-pair), 128]
        packs = []
        for arr in (h1, l1):
            a5 = arr.reshape(DK2, 2, P, FCH, P)        # [jp, i, p, fc, c]
            packs.append(a5.transpose(2, 0, 3, 1, 4).reshape(P, DK2, 2 * FCH, P))
        w1p.append(np.ascontiguousarray(np.concatenate(packs, axis=1)))
        h2, l2 = _quant_pair(w2_all[e], np.float32(64.0))
        w2cat = np.concatenate([
            h2.reshape(FCH, P, O), l2.reshape(FCH, P, O)], axis=0)
        w2p.append(np.ascontiguousarray(w2cat.transpose(1, 0, 2)))
        b1p.append(np.ascontiguousarray(
            (16.0 * b1_all[e]).reshape(FCH, P).T.astype(np.float32)))
    gwh, gwl = _quant_pair(gw, np.float32(16.0))
    gwpad = np.zeros((2 * DK, P, P), E4)               # [ktile, p, col(pad 128)]
    gwpad[:DK, :, :E] = gwh.reshape(DK, P, E)
    gwpad[DK:, :, :E] = gwl.reshape(DK, P, E)
    gwp = np.ascontiguousarray(gwpad.transpose(1, 0, 2))
    gbp = gb.reshape(E, 1).astype(np.float32)

    core_expert = [0, 0, 1, 1, 2, 3, 4, 5]
    core_tokens = [
        np.arange(0, half), np.arange(half, T),
        np.arange(0, half), np.arange(half, T),
    ] + [np.nonzero(keep[:, r])[0] for r in range(E)]

    def xpack(tl, src):
        arr = np.zeros((cap, D), E4)
        arr[:len(tl)] = src[tl]
        out = np.zeros((NBLK, P, DK, BMAX), E4)
        t0 = 0
        for i, tb in enumerate(blks):
            blkarr = arr[t0:t0 + tb].astype(np.float32)  # [tb, D]
            out[i, :, :, :tb] = blkarr.reshape(tb, DK, P).transpose(2, 1, 0)
            t0 += tb
        return out

    in_maps = []
    for c in range(NCORES):
        e = core_expert[c]
        tl = core_tokens[c]
        assert len(tl) <= cap
        sel = np.zeros((E, 2), np.float32)
        sel[:, 1] = 1.0
        if c < 4:
            sel[:, 0] = 0.5 * (2.0 ** -10)
        else:
            sel[c - 4, 0] = 2.0 ** -10
        in_maps.append({
            "xq": xpack(tl, xq8), "xlo": xpack(tl, xlo8),
            "w1": w1p[e], "w2": w2p[e], "b1": b1p[e],
            "gw": gwp, "gb": gbp, "sel": sel,
        })

    b1nz = bool(np.any(sb1) or np.any(rb1))
    nc = _get_nc(blks, b1nz)
    res = run_bass_kernel_spmd(nc, in_maps, list(range(NCORES)))

    out = np.zeros((T, O), np.float32)
    for c in range(NCORES):
        tl = core_tokens[c]
        yg = np.asarray(res.results[c]["yg"]).reshape(NBLK, BMAX, O)
        yg = np.concatenate([yg[i, :tb] for i, tb in enumerate(blks)], axis=0)
        out[tl] += yg[:len(tl)].astype(np.float32)
        # general-case second-layer bias would be added here (zero for this net)
    return out.reshape(B, S, O)
